# revision 1
# baseline (speedup 1.0000x reference)
"""Single-head causal self-attention (B=4, S=4096, D=512) on 8 trn2 NeuronCores.

Sharding: 2 cores per batch element. Each core handles ALL queries of its
batch but only the even- or odd-indexed 128-row KV tiles (flash-decoding
style KV-parity split). This keeps the SPMD program identical across cores,
perfectly load-balances the causal triangle, and wastes no tiles.

Each core emits unnormalized partial outputs po = (sum_k exp(s) * v) @ Wo^T
and partial row-sums l. Host combines: out = (po0 + po1) / (l0 + l1) + bo.
Scores/sqrt(D) are bounded (~N(0, 0.33)) for well-scaled inputs, so exp
without max-subtraction is safe; softmax is shift-invariant so the result
matches the reference.

Dataflow is fully transposed (x and weights passed pre-transposed, bf16) so
no on-device transposes are needed:
  Q^T[d,q]  = (WqT chunks)^T @ xT          (lhsT=WqT[e,d], rhs=xT[e,q])
  K^T[d,k]  = same with WkT, x-local
  V[k,d]    = (x-local chunks)^T @ WvT     (lhsT=xT[e,k], rhs=WvT[e,d])
  S^T[k,q]  = (K^T chunks)^T @ Q^T         accum over d
  P^T       = exp(S^T * scale)             (ScalarE, PSUM->SBUF bf16)
  l[1,q]    = ones^T @ P^T                 accum over k tiles
  A^T[d,q]  = (V chunks)^T @ P^T           accum over k tiles
  po[q,e]   = (A^T chunks)^T @ WoT         accum over d
"""

import numpy as np
import ml_dtypes

import concourse.bass as bass
import concourse.tile as tile
from concourse import bacc, mybir
from concourse import bass_utils

B, S, D = 4, 4096, 512
TS = 128              # kv tile rows
QB = 512              # query block
NQB = S // QB         # 8 query blocks
NLT = S // TS // 2    # 16 local kv tiles per core
NDC = D // 128        # 4 chunks of d
SL = S // 2           # 2048 local kv rows
SCALE = 1.0 / float(np.sqrt(D))
BF16 = mybir.dt.bfloat16
F32 = mybir.dt.float32
N_CORES = 8


def build_kernel(repeat=1):
    nc = bacc.Bacc("TRN2", target_bir_lowering=False, debug=False)

    xT = nc.dram_tensor("xT", [D, S], BF16, kind="ExternalInput").ap()
    xfl = nc.dram_tensor("xfl", [D, SL], BF16, kind="ExternalInput").ap()
    wqT = nc.dram_tensor("wqT", [D, D], BF16, kind="ExternalInput").ap()
    wkT = nc.dram_tensor("wkT", [D, D], BF16, kind="ExternalInput").ap()
    wvT = nc.dram_tensor("wvT", [D, D], BF16, kind="ExternalInput").ap()
    woT = nc.dram_tensor("woT", [D, D], BF16, kind="ExternalInput").ap()
    masks = nc.dram_tensor("masks", [2 * TS, QB], BF16, kind="ExternalInput").ap()
    po = nc.dram_tensor("po", [S, D], F32, kind="ExternalOutput").ap()
    lrow = nc.dram_tensor("lrow", [1, S], F32, kind="ExternalOutput").ap()

    with tile.TileContext(nc) as tc:
        with tc.tile_pool(name="persist", bufs=1) as P:
            xT_sb = [P.tile([TS, S], BF16, tag=f"xT{e}", name=f"xT{e}") for e in range(NDC)]
            xfl_sb = [P.tile([TS, SL], BF16, tag=f"xfl{e}", name=f"xfl{e}") for e in range(NDC)]
            w_sb = {
                wname: [P.tile([TS, D], BF16, tag=f"w{wname}{e}", name=f"w{wname}{e}")
                        for e in range(NDC)]
                for wname in ("q", "k", "v", "o")
            }
            # DMA issue order = first-consumption order: V-proj needs xfl+wvT
            # first, then wkT (K-proj), then xT+wqT (Q-proj), then woT.
            # xfl/xT are split into 512-col pieces so the first matmuls can
            # start as soon as the leading columns of every e-chunk land.
            for cb in range(SL // QB):
                for e in range(NDC):
                    nc.sync.dma_start(xfl_sb[e][:, cb * QB:(cb + 1) * QB],
                                      xfl[e * TS:(e + 1) * TS, cb * QB:(cb + 1) * QB])
                if cb == 0:
                    for e in range(NDC):
                        nc.sync.dma_start(w_sb["v"][e][:], wvT[e * TS:(e + 1) * TS, :])
                if cb == 1:
                    for e in range(NDC):
                        nc.sync.dma_start(w_sb["k"][e][:], wkT[e * TS:(e + 1) * TS, :])
            for cb in range(S // QB):
                for e in range(NDC):
                    nc.sync.dma_start(xT_sb[e][:, cb * QB:(cb + 1) * QB],
                                      xT[e * TS:(e + 1) * TS, cb * QB:(cb + 1) * QB])
                if cb == 0:
                    for e in range(NDC):
                        nc.sync.dma_start(w_sb["q"][e][:], wqT[e * TS:(e + 1) * TS, :])
                if cb == 1:
                    for e in range(NDC):
                        nc.sync.dma_start(w_sb["o"][e][:], woT[e * TS:(e + 1) * TS, :])
            mask_sb = [P.tile([TS, QB], BF16, tag=f"mask{c}", name=f"mask{c}") for c in range(2)]
            for c in range(2):
                nc.sync.dma_start(mask_sb[c][:], masks[c * TS:(c + 1) * TS, :])
            ones_sb = P.tile([TS, 1], BF16, tag="ones", name="ones")
            nc.gpsimd.memset(ones_sb[:], 1.0)

            QT_sb = [P.tile([TS, S], BF16, tag=f"QT{dc}", name=f"QT{dc}") for dc in range(NDC)]
            KT_sb = [P.tile([TS, SL], BF16, tag=f"KT{dc}", name=f"KT{dc}") for dc in range(NDC)]
            V_sb = [P.tile([TS, D], BF16, tag=f"V{kt}", name=f"V{kt}") for kt in range(NLT)]

            from contextlib import ExitStack
            with ExitStack() as rep_ctx:
                if repeat > 1:
                    rep_ctx.enter_context(tc.For_i(0, repeat, 1))
                # ---- projections ----
                with tc.tile_pool(name="proj_ps", bufs=4, space="PSUM") as PP:
                    # V[k,d]: lhsT = xfl[e, k-chunk], rhs = WvT[e, :]
                    for kt in range(NLT):
                        ps = PP.tile([TS, D], F32, tag="ps", name="ps_v")
                        for e in range(NDC):
                            nc.tensor.matmul(
                                ps[:], xfl_sb[e][:, kt * TS:(kt + 1) * TS], w_sb["v"][e][:],
                                start=(e == 0), stop=(e == NDC - 1))
                        nc.vector.tensor_copy(V_sb[kt][:], ps[:])
                    # K^T[d,k]: lhsT = WkT[e, d-chunk], rhs = xfl[e, colblock]
                    for dc in range(NDC):
                        for cb in range(SL // QB):
                            ps = PP.tile([TS, QB], F32, tag="ps", name="ps_p")
                            for e in range(NDC):
                                nc.tensor.matmul(
                                    ps[:], w_sb["k"][e][:, dc * TS:(dc + 1) * TS],
                                    xfl_sb[e][:, cb * QB:(cb + 1) * QB],
                                    start=(e == 0), stop=(e == NDC - 1))
                            nc.vector.tensor_copy(KT_sb[dc][:, cb * QB:(cb + 1) * QB], ps[:])
                    # Q^T[d,q]: lhsT = WqT[e, d-chunk], rhs = xT[e, colblock]
                    for dc in range(NDC):
                        for cb in range(S // QB):
                            ps = PP.tile([TS, QB], F32, tag="ps", name="ps_p")
                            for e in range(NDC):
                                nc.tensor.matmul(
                                    ps[:], w_sb["q"][e][:, dc * TS:(dc + 1) * TS],
                                    xT_sb[e][:, cb * QB:(cb + 1) * QB],
                                    start=(e == 0), stop=(e == NDC - 1))
                            nc.vector.tensor_copy(QT_sb[dc][:, cb * QB:(cb + 1) * QB], ps[:])

                # ---- attention + output projection, per query block ----
                with tc.tile_pool(name="st_ps", bufs=2, space="PSUM") as STP, \
                     tc.tile_pool(name="attn_ps", bufs=1, space="PSUM") as ATP, \
                     tc.tile_pool(name="l_ps", bufs=1, space="PSUM") as LP, \
                     tc.tile_pool(name="po_ps", bufs=1, space="PSUM") as POP, \
                     tc.tile_pool(name="p_sb", bufs=6) as PSB, \
                     tc.tile_pool(name="o_sb", bufs=3) as OSB:
                    for j in range(NQB):
                        nlt = 2 * j + 2
                        qcol = slice(j * QB, (j + 1) * QB)
                        attn_ps = [ATP.tile([TS, QB], F32, tag=f"attn{dc}", name=f"attn{dc}") for dc in range(NDC)]
                        l_ps = LP.tile([1, QB], F32, tag="l", name="l")
                        for lt in range(nlt):
                            st = STP.tile([TS, QB], F32, tag="st", name="st")
                            for dc in range(NDC):
                                nc.tensor.matmul(
                                    st[:], KT_sb[dc][:, lt * TS:(lt + 1) * TS], QT_sb[dc][:, qcol],
                                    start=(dc == 0), stop=(dc == NDC - 1))
                            p = PSB.tile([TS, QB], BF16, tag="p", name="p")
                            nc.scalar.activation(
                                p[:], st[:], mybir.ActivationFunctionType.Exp, scale=SCALE)
                            if lt >= 2 * j:
                                nc.vector.tensor_mul(p[:], p[:], mask_sb[lt - 2 * j][:])
                            nc.tensor.matmul(l_ps[:], ones_sb[:], p[:],
                                             start=(lt == 0), stop=(lt == nlt - 1))
                            for dc in range(NDC):
                                nc.tensor.matmul(
                                    attn_ps[dc][:], V_sb[lt][:, dc * TS:(dc + 1) * TS], p[:],
                                    start=(lt == 0), stop=(lt == nlt - 1))
                        l_sb = OSB.tile([1, QB], F32, tag="l_sb", name="l_sb")
                        nc.vector.tensor_copy(l_sb[:], l_ps[:])
                        nc.sync.dma_start(lrow[0:1, qcol], l_sb[:])
                        attn_sb = [OSB.tile([TS, QB], BF16, tag=f"attn_sb{dc}", name=f"attn_sb{dc}") for dc in range(NDC)]
                        for half in range(2):
                            hs = slice(half * (QB // 2), (half + 1) * (QB // 2))
                            for dc in range(NDC):
                                nc.vector.tensor_copy(attn_sb[dc][:, hs], attn_ps[dc][:, hs])
                        for qc in range(QB // TS):
                            po_ps = POP.tile([TS, D], F32, tag="po", name="po_ps_t")
                            for dc in range(NDC):
                                nc.tensor.matmul(
                                    po_ps[:], attn_sb[dc][:, qc * TS:(qc + 1) * TS], w_sb["o"][dc][:],
                                    start=(dc == 0), stop=(dc == NDC - 1))
                            po_sb = OSB.tile([TS, D], F32, tag="po_sb", name="po_sb")
                            nc.vector.tensor_copy(po_sb[:], po_ps[:])
                            r0 = j * QB + qc * TS
                            nc.sync.dma_start(po[r0:r0 + TS, :], po_sb[:])
    nc.compile()
    return nc


_cache = {}


def _make_masks(h):
    m = np.zeros((2 * TS, QB), dtype=np.float32)
    k_r = np.arange(TS)[:, None]
    q_r = np.arange(QB)[None, :]
    for c in range(2):
        m[c * TS:(c + 1) * TS] = (q_r >= 128 * (2 * c + h) + k_r)
    return m.astype(ml_dtypes.bfloat16)


def kernel(x, Wq, Wk, Wv, Wo, bo):
    bf = ml_dtypes.bfloat16
    x = np.asarray(x, dtype=np.float32)
    Wq, Wk, Wv, Wo, bo = (np.asarray(a, dtype=np.float32) for a in (Wq, Wk, Wv, Wo, bo))
    if "nc" not in _cache:
        _cache["nc"] = build_kernel()
    nc = _cache["nc"]

    wqT = np.ascontiguousarray(Wq.T).astype(bf)
    wkT = np.ascontiguousarray(Wk.T).astype(bf)
    wvT = np.ascontiguousarray(Wv.T).astype(bf)
    woT = np.ascontiguousarray(Wo.T).astype(bf)
    mask_h = [_make_masks(0), _make_masks(1)]

    # local kv columns for parity h: 128-col tiles with global tile index % 2 == h
    col_idx = {}
    for h in range(2):
        tiles = [np.arange(TS * (2 * lt + h), TS * (2 * lt + h) + TS) for lt in range(NLT)]
        col_idx[h] = np.concatenate(tiles)

    in_maps = []
    for core in range(N_CORES):
        b, h = core // 2, core % 2
        xTb = np.ascontiguousarray(x[b].T).astype(bf)     # [D, S]
        xflb = np.ascontiguousarray(xTb[:, col_idx[h]])
        in_maps.append({
            "xT": xTb, "xfl": xflb,
            "wqT": wqT, "wkT": wkT, "wvT": wvT, "woT": woT,
            "masks": mask_h[h],
        })

    global _last_in_maps
    _last_in_maps = in_maps
    res = bass_utils.run_bass_kernel_spmd(nc, in_maps, core_ids=list(range(N_CORES)))

    out = np.zeros((B, S, D), dtype=np.float32)
    for b in range(B):
        r0, r1 = res.results[2 * b], res.results[2 * b + 1]
        l = (r0["lrow"] + r1["lrow"]).reshape(S, 1)
        out[b] = (r0["po"] + r1["po"]) / l + bo.astype(np.float32)
    return out



# revision 2
# speedup vs baseline: 1.2205x; 1.2205x over previous
"""Single-head causal self-attention (B=4, S=4096, D=512) on 8 trn2 NeuronCores.

Sharding: 2 cores per batch element, KV-parity split (even/odd 128-row kv
tiles). Each core emits unnormalized partial outputs po and row-sums l;
host combines: out = (po0 + po1) / (l0 + l1) + bo.

v3: fully software-pipelined single PE stream.
- Scores in fp8e4 DoubleRow (2x contraction per matmul); Q^T/K^T quantized
  to fp8 by the projection PSUM->SBUF copies. Everything else bf16.
- No separate projection phase: the QKV/O projection chains for query
  block j+1 are interleaved into block j's attention groups as
  stall-absorbing PE filler (the repeat body recomputes block 0's
  projections at the end of block 7, feeding the next iteration; a
  prologue outside the loop seeds the first iteration).
- Depth-3 pipelining: scores run 3 groups ahead of l+attn, so exp (ACT)
  latency and the PSUM drains never stall the PE. po chains of block j-1
  are interleaved into block j.
- PSUM: st 1 bank + attn 4 + l 1 + scratch 2 (shared by proj and po
  chains) = 8 banks.
"""

import numpy as np
import ml_dtypes
from contextlib import ExitStack

import concourse.bass as bass
import concourse.tile as tile
from concourse import bacc, mybir
from concourse import bass_utils

B, S, D = 4, 4096, 512
TS = 128              # kv tile rows
QB = 512              # query block
NQB = S // QB         # 8 query blocks
NLT = S // TS // 2    # 16 local kv tiles per core
NDC = D // 128        # 4 chunks of d
SL = S // 2           # 2048 local kv rows
SCALE = 1.0 / float(np.sqrt(D))
BF16 = mybir.dt.bfloat16
F8 = mybir.dt.float8e4
F32 = mybir.dt.float32
N_CORES = 8
DR = mybir.MatmulPerfMode.DoubleRow
DEPTH = 3


def build_kernel(repeat=1):
    nc = bacc.Bacc("TRN2", target_bir_lowering=False, debug=False)

    xT = nc.dram_tensor("xT", [D, S], BF16, kind="ExternalInput").ap()
    xfl = nc.dram_tensor("xfl", [D, SL], BF16, kind="ExternalInput").ap()
    wqT = nc.dram_tensor("wqT", [D, D], BF16, kind="ExternalInput").ap()
    wkT = nc.dram_tensor("wkT", [D, D], BF16, kind="ExternalInput").ap()
    wvT = nc.dram_tensor("wvT", [D, D], BF16, kind="ExternalInput").ap()
    woT = nc.dram_tensor("woT", [D, D], BF16, kind="ExternalInput").ap()
    masks = nc.dram_tensor("masks", [2 * TS, QB], BF16, kind="ExternalInput").ap()
    po = nc.dram_tensor("po", [S, D], BF16, kind="ExternalOutput").ap()
    lrow = nc.dram_tensor("lrow", [1, S], F32, kind="ExternalOutput").ap()

    with tile.TileContext(nc) as tc:
        with tc.tile_pool(name="persist", bufs=1) as P:
            xT_sb = [P.tile([TS, S], BF16, tag=f"xT{e}", name=f"xT{e}") for e in range(NDC)]
            xfl_sb = [P.tile([TS, SL], BF16, tag=f"xfl{e}", name=f"xfl{e}") for e in range(NDC)]
            w_sb = {
                wname: [P.tile([TS, D], BF16, tag=f"w{wname}{e}", name=f"w{wname}{e}")
                        for e in range(NDC)]
                for wname in ("q", "k", "v", "o")
            }
            # DMA issue order = first-consumption order.
            for cb in range(SL // QB):
                for e in range(NDC):
                    nc.sync.dma_start(xfl_sb[e][:, cb * QB:(cb + 1) * QB],
                                      xfl[e * TS:(e + 1) * TS, cb * QB:(cb + 1) * QB])
                if cb == 0:
                    for e in range(NDC):
                        nc.sync.dma_start(w_sb["k"][e][:], wkT[e * TS:(e + 1) * TS, :])
                if cb == 1:
                    for e in range(NDC):
                        nc.sync.dma_start(w_sb["v"][e][:], wvT[e * TS:(e + 1) * TS, :])
            for cb in range(S // QB):
                for e in range(NDC):
                    nc.sync.dma_start(xT_sb[e][:, cb * QB:(cb + 1) * QB],
                                      xT[e * TS:(e + 1) * TS, cb * QB:(cb + 1) * QB])
                if cb == 0:
                    for e in range(NDC):
                        nc.sync.dma_start(w_sb["q"][e][:], wqT[e * TS:(e + 1) * TS, :])
                if cb == 1:
                    for e in range(NDC):
                        nc.sync.dma_start(w_sb["o"][e][:], woT[e * TS:(e + 1) * TS, :])
            mask_sb = [P.tile([TS, QB], BF16, tag=f"mask{c}", name=f"mask{c}") for c in range(2)]
            for c in range(2):
                nc.sync.dma_start(mask_sb[c][:], masks[c * TS:(c + 1) * TS, :])
            ones_sb = P.tile([TS, 1], BF16, tag="ones", name="ones")
            nc.gpsimd.memset(ones_sb[:], 1.0)

            # fp8 DoubleRow stores: logical d = c*256 + i*128 + p
            QT8 = [P.tile([TS, 2, S], F8, tag=f"QT8{c}", name=f"QT8{c}") for c in range(2)]
            KT8 = [P.tile([TS, 2, SL], F8, tag=f"KT8{c}", name=f"KT8{c}") for c in range(2)]
            V_sb = [P.tile([TS, D], BF16, tag=f"V{kt}", name=f"V{kt}") for kt in range(NLT)]

            copy_ctr = [0]

            def psum_drain(dst, src):
                """PSUM->SBUF copy, alternating DVE / ACT."""
                if copy_ctr[0] % 2 == 0:
                    nc.vector.tensor_copy(dst, src)
                else:
                    nc.scalar.activation(dst, src,
                                         mybir.ActivationFunctionType.Copy, scale=1.0)
                copy_ctr[0] += 1

            def proj_chain_thunks(SC, grp):
                """Projection chains for group `grp`: K cb=grp (grp<4),
                Q cb=grp, V kt=2*grp, 2*grp+1. Each thunk = one 4-matmul
                chain + drain copy."""
                thunks = []

                def k_chain(dc, cb):
                    def t():
                        ps = SC.tile([TS, QB], F32, tag="scr", name="scr_k")
                        for e in range(NDC):
                            nc.tensor.matmul(
                                ps[:], w_sb["k"][e][:, dc * TS:(dc + 1) * TS],
                                xfl_sb[e][:, cb * QB:(cb + 1) * QB],
                                start=(e == 0), stop=(e == NDC - 1))
                        psum_drain(KT8[dc // 2][:, dc % 2, cb * QB:(cb + 1) * QB], ps[:])
                    return t

                def q_chain(dc, cb):
                    def t():
                        ps = SC.tile([TS, QB], F32, tag="scr", name="scr_q")
                        for e in range(NDC):
                            nc.tensor.matmul(
                                ps[:], w_sb["q"][e][:, dc * TS:(dc + 1) * TS],
                                xT_sb[e][:, cb * QB:(cb + 1) * QB],
                                start=(e == 0), stop=(e == NDC - 1))
                        psum_drain(QT8[dc // 2][:, dc % 2, cb * QB:(cb + 1) * QB], ps[:])
                    return t

                def v_chain(kt):
                    def t():
                        ps = SC.tile([TS, D], F32, tag="scr", name="scr_v")
                        for e in range(NDC):
                            nc.tensor.matmul(
                                ps[:], xfl_sb[e][:, kt * TS:(kt + 1) * TS], w_sb["v"][e][:],
                                start=(e == 0), stop=(e == NDC - 1))
                        psum_drain(V_sb[kt][:], ps[:])
                    return t

                if grp < 4:
                    for dc in range(NDC):
                        thunks.append(k_chain(dc, grp))
                for dc in range(NDC):
                    thunks.append(q_chain(dc, grp))
                thunks.append(v_chain(2 * grp))
                thunks.append(v_chain(2 * grp + 1))
                return thunks

            with ExitStack() as rep_ctx:
                # Prologue: group 0 projections seed the first iteration.
                with tc.tile_pool(name="pro_ps", bufs=2, space="PSUM") as PRS:
                    for t in proj_chain_thunks(PRS, 0):
                        t()

                if repeat > 1:
                    rep_ctx.enter_context(tc.For_i(0, repeat, 1))

                with tc.tile_pool(name="st_ps", bufs=1, space="PSUM") as STP, \
                     tc.tile_pool(name="attn_ps", bufs=1, space="PSUM") as ATP, \
                     tc.tile_pool(name="l_ps", bufs=1, space="PSUM") as LP, \
                     tc.tile_pool(name="scr_ps", bufs=2, space="PSUM") as SC, \
                     tc.tile_pool(name="p_sb", bufs=6) as PSB, \
                     tc.tile_pool(name="o_sb", bufs=3) as OSB, \
                     tc.tile_pool(name="asb", bufs=2) as ASB:

                    def emit_scores(j, lt, st, qsl):
                        qcol = slice(j * QB + qsl.start, j * QB + qsl.stop)
                        for c in range(2):
                            nc.tensor.matmul(
                                st[:, qsl], KT8[c][:, :, lt * TS:(lt + 1) * TS],
                                QT8[c][:, :, qcol],
                                start=(c == 0), stop=(c == 1), perf_mode=DR)

                    def emit_po_chain(jp, qc, attn_sb_p):
                        po_ps = SC.tile([TS, D], F32, tag="scr", name="scr_po")
                        for dc in range(NDC):
                            nc.tensor.matmul(
                                po_ps[:], attn_sb_p[dc][:, qc * TS:(qc + 1) * TS],
                                w_sb["o"][dc][:],
                                start=(dc == 0), stop=(dc == NDC - 1))
                        po_sb = OSB.tile([TS, D], BF16, tag="po_sb", name="po_sb")
                        psum_drain(po_sb[:], po_ps[:])
                        r0 = jp * QB + qc * TS
                        nc.sync.dma_start(po[r0:r0 + TS, :], po_sb[:])

                    def emit_drain(jp, attn_ps_p, l_ps_p):
                        """PSUM->SBUF drains for block jp; halves ordered so po
                        chain qc=0/1 (cols 0:256 of every dc) unblocks first."""
                        attn_sb_p = [ASB.tile([TS, QB], BF16, tag=f"attn_sb{dc}",
                                              name=f"attn_sb{dc}") for dc in range(NDC)]
                        for half in range(2):
                            hs = slice(half * (QB // 2), (half + 1) * (QB // 2))
                            for dc in range(NDC):
                                psum_drain(attn_sb_p[dc][:, hs], attn_ps_p[dc][:, hs])
                        l_sb = OSB.tile([1, QB], F32, tag="l_sb", name="l_sb")
                        nc.vector.tensor_copy(l_sb[:], l_ps_p[:])
                        nc.sync.dma_start(lrow[0:1, jp * QB:(jp + 1) * QB], l_sb[:])
                        return attn_sb_p

                    prev = None  # (j, attn_ps, l_ps) of previous block
                    for j in range(NQB):
                        nlt = 2 * j + 2
                        order = [2 * j, 2 * j + 1] + list(range(2 * j))
                        qsl_of = {lt: slice(0, QB) for lt in order}
                        if j >= 1:
                            qsl_of[2 * j + 1] = slice(QB // 2, QB)

                        # proj filler: group j+1 during block j; block 7 carries
                        # group 0 for the next loop iteration, legal only after
                        # the scores/attn of tiles 0..3 have consumed the old
                        # KT8 cb0 / V0-1 (from idx 6 on).
                        filler = list(proj_chain_thunks(SC, (j + 1) % NQB))
                        fstart = 6 if j == NQB - 1 else 1
                        nslots = nlt - fstart
                        cpg = -(-len(filler) // max(1, nslots))

                        attn_ps = [ATP.tile([TS, QB], F32, tag=f"attn{dc}", name=f"attn{dc}")
                                   for dc in range(NDC)]
                        l_ps = LP.tile([1, QB], F32, tag="l", name="l")
                        p_of = {}
                        po_q = [0, 1, 2, 3] if prev is not None else []

                        def emit_lattn(j, lt, first, last):
                            qsl = qsl_of[lt]
                            p = p_of.pop(lt)
                            nc.tensor.matmul(l_ps[0:1, qsl], ones_sb[:], p[:, qsl],
                                             start=first, stop=last)
                            for dc in range(NDC):
                                nc.tensor.matmul(
                                    attn_ps[dc][:, qsl],
                                    V_sb[lt][:, dc * TS:(dc + 1) * TS], p[:, qsl],
                                    start=first, stop=last)

                        for idx, lt in enumerate(order):
                            # po/proj filler first: covers the st-bank WAR on
                            # exp(prev) and the drain-copy wait of po chains
                            if idx >= 1 and po_q:
                                emit_po_chain(jp, po_q.pop(0), attn_sb_p)
                            if idx >= fstart:
                                for _ in range(cpg):
                                    if filler:
                                        filler.pop(0)()
                            qsl = qsl_of[lt]
                            st = STP.tile([TS, QB], F32, tag="st", name="st")
                            emit_scores(j, lt, st, qsl)
                            p = PSB.tile([TS, QB], BF16, tag="p", name="p")
                            p_of[lt] = p
                            nc.scalar.activation(
                                p[:, qsl], st[:, qsl],
                                mybir.ActivationFunctionType.Exp, scale=SCALE)
                            if lt == 2 * j or lt == 2 * j + 1:
                                ms = mask_sb[lt - 2 * j]
                                nc.vector.tensor_mul(p[:, qsl], p[:, qsl], ms[:, qsl])
                            if idx == 0 and prev is not None:
                                jp, attn_ps_p, l_ps_p = prev
                                attn_sb_p = emit_drain(jp, attn_ps_p, l_ps_p)
                            if idx >= DEPTH:
                                plt = order[idx - DEPTH]
                                emit_lattn(j, plt, first=(idx == DEPTH), last=False)
                        # tail: leftover filler/po + last DEPTH l+attn groups
                        tail0 = max(0, nlt - DEPTH)
                        for pos in range(tail0, nlt):
                            if po_q:
                                emit_po_chain(jp, po_q.pop(0), attn_sb_p)
                            if filler:
                                filler.pop(0)()
                            emit_lattn(j, order[pos], first=(pos == 0),
                                       last=(pos == nlt - 1))
                        while filler:
                            filler.pop(0)()
                        prev = (j, attn_ps, l_ps)

                    # final block drain + po
                    jp, attn_ps_p, l_ps_p = prev
                    attn_sb_p = emit_drain(jp, attn_ps_p, l_ps_p)
                    for qc in range(4):
                        emit_po_chain(jp, qc, attn_sb_p)
    nc.compile()
    return nc


_cache = {}


def _make_masks(h):
    m = np.zeros((2 * TS, QB), dtype=np.float32)
    k_r = np.arange(TS)[:, None]
    q_r = np.arange(QB)[None, :]
    for c in range(2):
        m[c * TS:(c + 1) * TS] = (q_r >= 128 * (2 * c + h) + k_r)
    return m.astype(ml_dtypes.bfloat16)


def kernel(x, Wq, Wk, Wv, Wo, bo):
    bf = ml_dtypes.bfloat16
    x = np.asarray(x, dtype=np.float32)
    Wq, Wk, Wv, Wo, bo = (np.asarray(a, dtype=np.float32) for a in (Wq, Wk, Wv, Wo, bo))
    if "nc" not in _cache:
        _cache["nc"] = build_kernel()
    nc = _cache["nc"]

    wqT = np.ascontiguousarray(Wq.T).astype(bf)
    wkT = np.ascontiguousarray(Wk.T).astype(bf)
    wvT = np.ascontiguousarray(Wv.T).astype(bf)
    woT = np.ascontiguousarray(Wo.T).astype(bf)
    mask_h = [_make_masks(0), _make_masks(1)]

    col_idx = {}
    for h in range(2):
        tiles = [np.arange(TS * (2 * lt + h), TS * (2 * lt + h) + TS) for lt in range(NLT)]
        col_idx[h] = np.concatenate(tiles)

    in_maps = []
    for core in range(N_CORES):
        b, h = core // 2, core % 2
        xTb = np.ascontiguousarray(x[b].T).astype(bf)     # [D, S]
        xflb = np.ascontiguousarray(xTb[:, col_idx[h]])
        in_maps.append({
            "xT": xTb, "xfl": xflb,
            "wqT": wqT, "wkT": wkT, "wvT": wvT, "woT": woT,
            "masks": mask_h[h],
        })

    global _last_in_maps
    _last_in_maps = in_maps
    res = bass_utils.run_bass_kernel_spmd(nc, in_maps, core_ids=list(range(N_CORES)))

    out = np.zeros((B, S, D), dtype=np.float32)
    for b in range(B):
        r0, r1 = res.results[2 * b], res.results[2 * b + 1]
        l = (r0["lrow"] + r1["lrow"]).reshape(S, 1)
        out[b] = (r0["po"].astype(np.float32) + r1["po"].astype(np.float32)) / l + bo.astype(np.float32)
    return out
